# revision 18
# baseline (speedup 1.0000x reference)
"""LoRA LayerNorm Trainium2 kernel (8-core data-parallel, raw Bass).

out = x_hat * scale + shift, where
  x_hat    = (x - mean) * rsqrt(var + eps)        (LayerNorm over last dim)
  scale[i] = sum_r A_s[i,r] * B_s[r,i] * 2.0      (low-rank diagonal)
  shift[i] = sum_r A_h[i,r] * B_h[r,i] * 2.0

The tiny [N,4] LoRA diagonals are folded on the host (64K FLOPs); the
device kernel receives scale_vec/shift_vec [N] (bf16) and x shards
[1024, N] (f32).

Per-core algorithm (rows on partitions, 8 tiles of [128, 8192], x
quad-buffered so load/store DMA hides behind compute):
  setup: scale/shift rows land in partition 0 of their broadcast tiles;
         the otherwise-idle PE replicates them across all 128 partitions
         via ones-matmuls into PSUM and ACT evacuates (keeps the 4MB
         broadcast off the DMA engines; bf16 round-trip is exact).
  ACT (iter t): std(t-1) = Sqrt(u/N + eps), then two full-width passes
        for tile t: sx = sum(x) via Copy+accum_out and sq = sum(x^2)
        via Square+accum_out (bf16 garbage sink; fp32 accumulators).
        Output stores ride the end of each iteration, where their p2
        wait is already satisfied.
  DVE (iter t): pass1 chunks tb = (x + (-mean)) * scale_bc (tb bf16 in
        SBUF), then pass2 chunks x = (tb * rstd) + shift_bc in-place
        into the x buffer. Tiny ops (nm = -sx/N, u = sx*nm + sq = N*var,
        rstd = 1/std) are interleaved >=1 big op from their producers,
        so no same-engine RAW drains are needed.
  SYNC: x tile loads only (a store's p2 wait must never block a load
        issue on an in-order queue).
var = (sum(x^2) - sum(x)^2/N)/N is safe here (x ~ N(0,1), var ~ 1).
bf16 for scale/shift/tb costs ~2.4e-3 relative error vs the 2e-2 gate.
"""

import numpy as np
import ml_dtypes
from contextlib import ExitStack

import concourse.bass as bass
from concourse import mybir
from concourse.bass_utils import run_bass_kernel_spmd

F32 = mybir.dt.float32
BF16 = mybir.dt.bfloat16

# Problem geometry (hardcoded; see module docstring)
B_DIM, S_DIM, N = 2, 4096, 8192
RANK = 4
SCALING = 2.0  # alpha / rank = 8 / 4
EPS = 1e-5
NCORES = 8
ROWS = B_DIM * S_DIM // NCORES  # 1024 rows per core
P = 128
NTILES = ROWS // P              # 8
CHUNK = 2048                    # evac chunk width (psum-bank bound)
TCH = 4096                      # STT transform chunk width
NCHUNK = N // CHUNK             # 4
HALF = N // 2                   # tail store granularity
NBUF = 4                        # x tile buffers


def build_nc() -> bass.Bass:
    nc = bass.Bass()

    x = nc.declare_dram_parameter("x_shard", [ROWS, N], F32, isOutput=False)
    sv = nc.declare_dram_parameter("scale_vec", [N], BF16, isOutput=False)
    hv = nc.declare_dram_parameter("shift_vec", [N], BF16, isOutput=False)
    y = nc.declare_dram_parameter("y_shard", [ROWS, N], F32, isOutput=True)

    with ExitStack() as ctx:
        ec = ctx.enter_context
        # big tiles: 4x32(xb) + 16(garb) + 16(tb) + 2x16(bc) = 192 KiB/part
        xb = [ec(nc.sbuf_tensor(f"xb{i}", [P, N], F32)) for i in range(NBUF)]
        garb = ec(nc.sbuf_tensor("garb", [P, N], BF16))  # ACT accum sink
        tb = ec(nc.sbuf_tensor("tb", [P, N], BF16))      # pass1 output
        scale_bc = ec(nc.sbuf_tensor("scale_bc", [P, N], BF16))
        shift_bc = ec(nc.sbuf_tensor("shift_bc", [P, N], BF16))
        # PE broadcast staging (setup only)
        tbp = ec(nc.psum_tensor("tbp", [P, 2 * CHUNK], F32))
        # per-tile stats scalars
        sx_ = [ec(nc.sbuf_tensor(f"sx{i}", [P, 1], F32)) for i in range(NBUF)]
        sq_ = [ec(nc.sbuf_tensor(f"sq{i}", [P, 1], F32)) for i in range(NBUF)]
        u_ = [ec(nc.sbuf_tensor(f"u{i}", [P, 1], F32)) for i in range(2)]
        nm_ = [ec(nc.sbuf_tensor(f"nm{i}", [P, 1], F32)) for i in range(2)]
        std_ = [ec(nc.sbuf_tensor(f"std{i}", [P, 1], F32)) for i in range(2)]
        rstd_ = [ec(nc.sbuf_tensor(f"rstd{i}", [P, 1], F32)) for i in range(2)]
        zt = ec(nc.sbuf_tensor("zt", [P, 1], F32))
        eps_t = ec(nc.sbuf_tensor("eps_t", [P, 1], F32))
        ones_t = ec(nc.sbuf_tensor("ones_t", [1, P], BF16))

        sems = {}
        for s in (
            "load0", "load1", "load2", "load3",
            "store0", "store1", "store2", "store3",
            "rows", "pe", "evac", "acc", "vv", "std", "p2", "const",
        ):
            sems[s] = ec(nc.semaphore(s))
        loadS = [sems[f"load{i}"] for i in range(NBUF)]
        storeS = [sems[f"store{i}"] for i in range(NBUF)]

        with nc.Block() as block:

            @block.sync
            def _(sp):
                # scale/shift rows into partition 0 of the bc tiles
                for vec, dst in ((sv, scale_bc), (hv, shift_bc)):
                    sp.dma_start(
                        out=dst[0:1, :],
                        in_=vec[:].rearrange("(u n) -> u n", u=1),
                    ).then_inc(sems["rows"], 16)
                for t in range(NTILES):
                    b = t % NBUF
                    if t >= NBUF:
                        # xb[b] free for reload once tile t-NBUF retired
                        sp.wait_ge(storeS[b], 16 * (t // NBUF))
                    sp.dma_start(
                        out=xb[b][:], in_=x[t * P:(t + 1) * P, :]
                    ).then_inc(loadS[b], 16)

            @block.tensor
            def _(te):
                # replicate scale/shift rows across all 128 partitions:
                # psum_slice[p, f] = ones[p] * row[f]
                te.wait_ge(sems["const"], 3)
                for vi, dst in enumerate((scale_bc, shift_bc)):
                    te.wait_ge(sems["rows"], 16 * (vi + 1))
                    for s in range(16):
                        g = vi * 4 + s // 4
                        if g >= 2 and s % 4 == 0:
                            # psum bank group g%2 free once chunk g-2 evac'd
                            te.wait_ge(sems["evac"], g - 1)
                        off = (g % 2) * CHUNK + (s % 4) * 512
                        nc.tensor.matmul(
                            tbp[:, off:off + 512],
                            ones_t[:],
                            dst[0:1, s * 512:(s + 1) * 512],
                            start=True,
                            stop=True,
                        ).then_inc(sems["pe"], 1)

            @block.scalar
            def _(sc):
                def store(u, half=False):
                    b = u % NBUF
                    if half:
                        sc.wait_ge(sems["p2"], 2 * u + 1)
                        sc.dma_start(
                            out=y[u * P:(u + 1) * P, 0:HALF],
                            in_=xb[b][:, 0:HALF],
                        ).then_inc(storeS[b], 16)
                        sc.wait_ge(sems["p2"], 2 * u + 2)
                        sc.dma_start(
                            out=y[u * P:(u + 1) * P, HALF:N],
                            in_=xb[b][:, HALF:N],
                        ).then_inc(storeS[b], 16)
                    else:
                        sc.wait_ge(sems["p2"], 2 * u + 2)
                        sc.dma_start(
                            out=y[u * P:(u + 1) * P, :], in_=xb[b][:]
                        ).then_inc(storeS[b], 16)

                def evac(c):
                    # partition 0 rewritten with identical bytes (exact)
                    dst = (scale_bc, shift_bc)[c // 4]
                    sc.wait_ge(sems["pe"], 4 * (c + 1))
                    sc.activation(
                        out=dst[:, (c % 4) * CHUNK:(c % 4 + 1) * CHUNK],
                        in_=tbp[:, (c % 2) * CHUNK:(c % 2 + 1) * CHUNK],
                        func=mybir.ActivationFunctionType.Copy,
                        bias=0.0,
                    ).then_inc(sems["evac"], 1)

                def stats(t):
                    b = t % NBUF
                    sc.wait_ge(loadS[b], 16 * (t // NBUF + 1))
                    sc.activation(
                        out=garb[:],
                        in_=xb[b][:],
                        func=mybir.ActivationFunctionType.Copy,
                        bias=0.0,
                        accum_out=sx_[b][:],
                    )
                    sc.activation(
                        out=garb[:],
                        in_=xb[b][:],
                        func=mybir.ActivationFunctionType.Square,
                        bias=zt[:],
                        accum_out=sq_[b][:],
                    ).then_inc(sems["acc"], 1)

                def std(t):
                    sc.wait_ge(sems["vv"], t + 1)
                    sc.activation(
                        out=std_[t % 2][:],
                        in_=u_[t % 2][:],
                        func=mybir.ActivationFunctionType.Sqrt,
                        bias=eps_t[:],
                        scale=1.0 / N,
                    ).then_inc(sems["std"], 1)

                sc.wait_ge(sems["const"], 2)
                for c in range(4):
                    evac(c)          # scale_bc (PE runs ahead)
                stats(0)             # overlaps PE's shift replication
                for c in range(4, 8):
                    evac(c)          # shift_bc
                for t in range(1, NTILES):
                    std(t - 1)
                    stats(t)
                    if t >= 2:
                        store(t - 2)
                std(NTILES - 1)
                store(NTILES - 2, half=True)
                store(NTILES - 1, half=True)

            @block.vector
            def _(v):
                v.memset(zt[:], 0.0).then_inc(sems["const"], 1)
                v.memset(eps_t[:], EPS).then_inc(sems["const"], 1)
                v.memset(ones_t[:], 1.0).then_inc(sems["const"], 1)

                for t in range(NTILES + 1):
                    w = t - 1          # tile being transformed
                    b = w % NBUF
                    p = w % 2

                    def p1(c):
                        sl = slice(c * TCH, (c + 1) * TCH)
                        v.scalar_tensor_tensor(
                            out=tb[:, sl],
                            in0=xb[b][:, sl],
                            scalar=nm_[p][:],
                            in1=scale_bc[:, sl],
                            op0=mybir.AluOpType.add,
                            op1=mybir.AluOpType.mult,
                        )

                    def p2(c):
                        sl = slice(c * TCH, (c + 1) * TCH)
                        v.scalar_tensor_tensor(
                            out=xb[b][:, sl],
                            in0=tb[:, sl],
                            scalar=rstd_[p][:],
                            in1=shift_bc[:, sl],
                            op0=mybir.AluOpType.mult,
                            op1=mybir.AluOpType.add,
                        ).then_inc(sems["p2"], 1)

                    def nm(tt):
                        v.tensor_scalar_mul(
                            nm_[tt % 2][:], sx_[tt % NBUF][:], -1.0 / N
                        )

                    def uu(tt):
                        # u = sx*nm + sq = sq - sx^2/N  ( = N*var )
                        v.scalar_tensor_tensor(
                            out=u_[tt % 2][:],
                            in0=sx_[tt % NBUF][:],
                            scalar=nm_[tt % 2][:],
                            in1=sq_[tt % NBUF][:],
                            op0=mybir.AluOpType.mult,
                            op1=mybir.AluOpType.add,
                        ).then_inc(sems["vv"], 1)

                    if t == 0:
                        v.wait_ge(sems["acc"], 1)
                        nm(0)
                        v.drain()  # cheap: pipe holds only tiny ops
                        uu(0)
                        continue
                    if w == 0:
                        v.wait_ge(sems["evac"], 4)   # scale_bc resident
                    # transforms(w) in two 4096-wide chunks per pass;
                    # tiny ops >=1 big STT downstream of their producers
                    p1(0)
                    v.wait_ge(sems["std"], w + 1)
                    v.reciprocal(rstd_[p][:], std_[p][:])
                    p1(1)
                    if w == 0:
                        v.wait_ge(sems["evac"], 8)   # shift_bc resident
                    p2(0)
                    if t < NTILES:
                        v.wait_ge(sems["acc"], t + 1)
                        nm(t)
                    p2(1)
                    if t < NTILES:
                        uu(t)

    return nc


def _prep(x, lora_scale_A, lora_scale_B, lora_shift_A, lora_shift_B):
    x = np.ascontiguousarray(np.asarray(x, dtype=np.float32).reshape(-1, N))
    scale = np.einsum(
        "nr,rn->n",
        np.asarray(lora_scale_A, np.float32),
        np.asarray(lora_scale_B, np.float32),
    ) * SCALING
    shift = np.einsum(
        "nr,rn->n",
        np.asarray(lora_shift_A, np.float32),
        np.asarray(lora_shift_B, np.float32),
    ) * SCALING
    args = {
        "scale_vec": np.ascontiguousarray(scale.astype(ml_dtypes.bfloat16)),
        "shift_vec": np.ascontiguousarray(shift.astype(ml_dtypes.bfloat16)),
    }
    return [
        {"x_shard": x[i * ROWS:(i + 1) * ROWS], **args} for i in range(NCORES)
    ]


def kernel(x, lora_scale_A, lora_scale_B, lora_shift_A, lora_shift_B):
    in_maps = _prep(x, lora_scale_A, lora_scale_B, lora_shift_A, lora_shift_B)
    nc = build_nc()
    res = run_bass_kernel_spmd(nc, in_maps, core_ids=list(range(NCORES)))
    out = np.concatenate(
        [res.results[i]["y_shard"] for i in range(NCORES)], axis=0
    )
    return out.reshape(B_DIM, S_DIM, N)


if __name__ == "__main__":
    import reference

    inputs = {k: np.asarray(v) for k, v in reference.setup_inputs().items()}
    expected = np.asarray(reference.reference(**inputs))
    actual = kernel(**inputs)
    err = np.abs(actual - expected)
    denom = np.abs(expected).max()
    print("max abs err:", err.max(), "rel:", err.max() / denom)


# revision 21
# speedup vs baseline: 1.0460x; 1.0460x over previous
"""LoRA LayerNorm Trainium2 kernel (8-core data-parallel, raw Bass).

out = x_hat * scale + shift, where
  x_hat    = (x - mean) * rsqrt(var + eps)        (LayerNorm over last dim)
  scale[i] = sum_r A_s[i,r] * B_s[r,i] * 2.0      (low-rank diagonal)
  shift[i] = sum_r A_h[i,r] * B_h[r,i] * 2.0

The tiny [N,4] LoRA diagonals are folded on the host (64K FLOPs); the
device kernel receives scale_vec/shift_vec [N] (bf16) and x shards
[1024, N] (f32).

Per-core algorithm (rows on partitions, 8 tiles of [128, 8192], x
quad-buffered so load/store DMA hides behind compute):
  setup: scale/shift rows land in partition 0 of their broadcast tiles;
         the otherwise-idle PE replicates them across all 128 partitions
         via ones-matmuls into PSUM and ACT evacuates (keeps the 4MB
         broadcast off the DMA engines; bf16 round-trip is exact).
  ACT (iter t): std(t-1) = Sqrt(u/N + eps), then two full-width passes
        for tile t: sx = sum(x) via Copy+accum_out and sq = sum(x^2)
        via Square+accum_out (bf16 garbage sink; fp32 accumulators).
        Output stores ride the end of each iteration, where their p2
        wait is already satisfied.
  DVE (iter t): pass1 chunks tb = (x + (-mean)) * scale_bc (tb bf16 in
        SBUF), then pass2 chunks x = (tb * rstd) + shift_bc in-place
        into the x buffer. Tiny ops (nm = -sx/N, u = sx*nm + sq = N*var,
        rstd = 1/std) are interleaved >=1 big op from their producers,
        so no same-engine RAW drains are needed.
  SYNC: x tile loads only (a store's p2 wait must never block a load
        issue on an in-order queue).
var = (sum(x^2) - sum(x)^2/N)/N is safe here (x ~ N(0,1), var ~ 1).
bf16 for scale/shift/tb costs ~2.4e-3 relative error vs the 2e-2 gate.
"""

import numpy as np
import ml_dtypes
from contextlib import ExitStack

import concourse.bass as bass
from concourse import mybir
from concourse.bass_utils import run_bass_kernel_spmd

F32 = mybir.dt.float32
BF16 = mybir.dt.bfloat16

# Problem geometry (hardcoded; see module docstring)
B_DIM, S_DIM, N = 2, 4096, 8192
RANK = 4
SCALING = 2.0  # alpha / rank = 8 / 4
EPS = 1e-5
NCORES = 8
ROWS = B_DIM * S_DIM // NCORES  # 1024 rows per core
P = 128
NTILES = ROWS // P              # 8
CHUNK = 2048                    # STT / evac chunk width
NCHUNK = N // CHUNK             # 4
HALF = N // 2                   # tail store granularity
NBUF = 4                        # x tile buffers


def build_nc() -> bass.Bass:
    nc = bass.Bass()

    x = nc.declare_dram_parameter("x_shard", [ROWS, N], F32, isOutput=False)
    sv = nc.declare_dram_parameter("scale_vec", [N], BF16, isOutput=False)
    hv = nc.declare_dram_parameter("shift_vec", [N], BF16, isOutput=False)
    y = nc.declare_dram_parameter("y_shard", [ROWS, N], F32, isOutput=True)

    with ExitStack() as ctx:
        ec = ctx.enter_context
        # big tiles: 4x32(xb) + 16(garb) + 16(tb) + 2x16(bc) = 192 KiB/part
        xb = [ec(nc.sbuf_tensor(f"xb{i}", [P, N], F32)) for i in range(NBUF)]
        garb = ec(nc.sbuf_tensor("garb", [P, N], BF16))  # ACT accum sink
        tb = ec(nc.sbuf_tensor("tb", [P, N], BF16))      # pass1 output
        scale_bc = ec(nc.sbuf_tensor("scale_bc", [P, N], BF16))
        shift_bc = ec(nc.sbuf_tensor("shift_bc", [P, N], BF16))
        # PE broadcast staging (setup only)
        tbp = ec(nc.psum_tensor("tbp", [P, 2 * CHUNK], F32))
        # per-tile stats scalars
        sx_ = [ec(nc.sbuf_tensor(f"sx{i}", [P, 1], F32)) for i in range(NBUF)]
        sq_ = [ec(nc.sbuf_tensor(f"sq{i}", [P, 1], F32)) for i in range(NBUF)]
        u_ = [ec(nc.sbuf_tensor(f"u{i}", [P, 1], F32)) for i in range(2)]
        nm_ = [ec(nc.sbuf_tensor(f"nm{i}", [P, 1], F32)) for i in range(2)]
        std_ = [ec(nc.sbuf_tensor(f"std{i}", [P, 1], F32)) for i in range(2)]
        rstd_ = [ec(nc.sbuf_tensor(f"rstd{i}", [P, 1], F32)) for i in range(2)]
        zt = ec(nc.sbuf_tensor("zt", [P, 1], F32))
        dacc = ec(nc.sbuf_tensor("dacc", [P, 1], F32))
        eps_t = ec(nc.sbuf_tensor("eps_t", [P, 1], F32))
        ones_t = ec(nc.sbuf_tensor("ones_t", [1, P], BF16))

        sems = {}
        for s in (
            "load0", "load1", "load2", "load3",
            "store0", "store1", "store2", "store3",
            "rows", "pe", "evac", "acc", "vv", "std", "p2", "const",
            "rs", "nmu",
        ):
            sems[s] = ec(nc.semaphore(s))
        loadS = [sems[f"load{i}"] for i in range(NBUF)]
        storeS = [sems[f"store{i}"] for i in range(NBUF)]

        with nc.Block() as block:

            @block.sync
            def _(sp):
                # scale/shift rows into partition 0 of the bc tiles
                for vec, dst in ((sv, scale_bc), (hv, shift_bc)):
                    sp.dma_start(
                        out=dst[0:1, :],
                        in_=vec[:].rearrange("(u n) -> u n", u=1),
                    ).then_inc(sems["rows"], 16)
                for t in range(NTILES):
                    b = t % NBUF
                    if t >= NBUF:
                        # xb[b] free for reload once tile t-NBUF retired
                        sp.wait_ge(storeS[b], 16 * (t // NBUF))
                    sp.dma_start(
                        out=xb[b][:], in_=x[t * P:(t + 1) * P, :]
                    ).then_inc(loadS[b], 16)

            @block.tensor
            def _(te):
                # replicate scale/shift rows across all 128 partitions:
                # psum_slice[p, f] = ones[p] * row[f]
                te.wait_ge(sems["const"], 3)
                for vi, dst in enumerate((scale_bc, shift_bc)):
                    te.wait_ge(sems["rows"], 16 * (vi + 1))
                    for s in range(16):
                        g = vi * 4 + s // 4
                        if g >= 2 and s % 4 == 0:
                            # psum bank group g%2 free once chunk g-2 evac'd
                            te.wait_ge(sems["evac"], g - 1)
                        off = (g % 2) * CHUNK + (s % 4) * 512
                        nc.tensor.matmul(
                            tbp[:, off:off + 512],
                            ones_t[:],
                            dst[0:1, s * 512:(s + 1) * 512],
                            start=True,
                            stop=True,
                        ).then_inc(sems["pe"], 1)

            @block.scalar
            def _(sc):
                def store(u, half=False):
                    b = u % NBUF
                    if half:
                        sc.wait_ge(sems["p2"], 2 * u + 1)
                        sc.dma_start(
                            out=y[u * P:(u + 1) * P, 0:HALF],
                            in_=xb[b][:, 0:HALF],
                        ).then_inc(storeS[b], 16)
                        sc.wait_ge(sems["p2"], 2 * u + 2)
                        sc.dma_start(
                            out=y[u * P:(u + 1) * P, HALF:N],
                            in_=xb[b][:, HALF:N],
                        ).then_inc(storeS[b], 16)
                    else:
                        sc.wait_ge(sems["p2"], 2 * u + 2)
                        sc.dma_start(
                            out=y[u * P:(u + 1) * P, :], in_=xb[b][:]
                        ).then_inc(storeS[b], 16)

                def evac(c):
                    # partition 0 rewritten with identical bytes (exact)
                    dst = (scale_bc, shift_bc)[c // 4]
                    sc.wait_ge(sems["pe"], 4 * (c + 1))
                    sc.activation(
                        out=dst[:, (c % 4) * CHUNK:(c % 4 + 1) * CHUNK],
                        in_=tbp[:, (c % 2) * CHUNK:(c % 2 + 1) * CHUNK],
                        func=mybir.ActivationFunctionType.Copy,
                        bias=0.0,
                    ).then_inc(sems["evac"], 1)

                def stats(t):
                    b = t % NBUF
                    sc.wait_ge(loadS[b], 16 * (t // NBUF + 1))
                    sc.activation(
                        out=garb[:],
                        in_=xb[b][:],
                        func=mybir.ActivationFunctionType.Copy,
                        bias=0.0,
                        accum_out=sx_[b][:],
                    )
                    sc.activation(
                        out=garb[:],
                        in_=xb[b][:],
                        func=mybir.ActivationFunctionType.Square,
                        bias=zt[:],
                        accum_out=sq_[b][:],
                    )
                    # accum_out lands via a separate READ_ACCUMULATOR
                    # instruction after each ACTIVATE; signal readiness
                    # from a trailing op so in-order retirement covers it
                    sc.activation(
                        out=dacc[:],
                        in_=zt[:],
                        func=mybir.ActivationFunctionType.Copy,
                        bias=0.0,
                    ).then_inc(sems["acc"], 1)

                def std(t):
                    sc.wait_ge(sems["vv"], t + 1)
                    sc.activation(
                        out=std_[t % 2][:],
                        in_=u_[t % 2][:],
                        func=mybir.ActivationFunctionType.Sqrt,
                        bias=eps_t[:],
                        scale=1.0 / N,
                    ).then_inc(sems["std"], 1)

                sc.wait_ge(sems["const"], 2)
                for c in range(4):
                    evac(c)          # scale_bc (PE runs ahead)
                stats(0)             # overlaps PE's shift replication
                for c in range(4, 8):
                    evac(c)          # shift_bc
                for t in range(1, NTILES):
                    std(t - 1)
                    stats(t)
                    if t >= 2:
                        store(t - 2)
                std(NTILES - 1)
                store(NTILES - 2, half=True)
                store(NTILES - 1, half=True)

            @block.vector
            def _(v):
                v.memset(zt[:], 0.0).then_inc(sems["const"], 1)
                v.memset(eps_t[:], EPS).then_inc(sems["const"], 1)
                v.memset(ones_t[:], 1.0).then_inc(sems["const"], 1)

                for t in range(NTILES + 1):
                    w = t - 1          # tile being transformed
                    b = w % NBUF
                    p = w % 2

                    def p1(c):
                        sl = slice(c * CHUNK, (c + 1) * CHUNK)
                        v.scalar_tensor_tensor(
                            out=tb[:, sl],
                            in0=xb[b][:, sl],
                            scalar=nm_[p][:],
                            in1=scale_bc[:, sl],
                            op0=mybir.AluOpType.add,
                            op1=mybir.AluOpType.mult,
                        )

                    def p2(c):
                        sl = slice(c * CHUNK, (c + 1) * CHUNK)
                        ins = v.scalar_tensor_tensor(
                            out=xb[b][:, sl],
                            in0=tb[:, sl],
                            scalar=rstd_[p][:],
                            in1=shift_bc[:, sl],
                            op0=mybir.AluOpType.mult,
                            op1=mybir.AluOpType.add,
                        )
                        if c % 2 == 1:
                            ins.then_inc(sems["p2"], 1)

                    def nm(tt):
                        v.tensor_scalar_mul(
                            nm_[tt % 2][:], sx_[tt % NBUF][:], -1.0 / N
                        ).then_inc(sems["nmu"], 1)

                    def uu(tt):
                        # u = sx*nm + sq = sq - sx^2/N  ( = N*var )
                        v.scalar_tensor_tensor(
                            out=u_[tt % 2][:],
                            in0=sx_[tt % NBUF][:],
                            scalar=nm_[tt % 2][:],
                            in1=sq_[tt % NBUF][:],
                            op0=mybir.AluOpType.mult,
                            op1=mybir.AluOpType.add,
                        ).then_inc(sems["vv"], 1)

                    if t == 0:
                        v.wait_ge(sems["acc"], 1)
                        nm(0)
                        v.wait_ge(sems["nmu"], 1)  # nm committed
                        uu(0)
                        continue
                    if w == 0:
                        v.wait_ge(sems["evac"], 4)   # scale_bc resident
                    # transforms(w) with stats(t) tiny ops at the end.
                    # Same-engine RAW on [P,1] scalars is closed with
                    # self-semaphores (producer then_inc -> consumer wait),
                    # not instruction distance: the engine pipeline gives
                    # no write->read interlock guarantee.
                    v.wait_ge(sems["nmu"], w + 1)    # nm(w) committed
                    p1(0)
                    v.wait_ge(sems["std"], w + 1)
                    v.reciprocal(rstd_[p][:], std_[p][:]).then_inc(
                        sems["rs"], 1
                    )
                    p1(1)
                    p1(2)
                    p1(3)
                    if w == 0:
                        v.wait_ge(sems["evac"], 8)   # shift_bc resident
                    v.wait_ge(sems["rs"], w + 1)     # rstd(w) committed
                    p2(0)
                    p2(1)
                    p2(2)
                    if t < NTILES:
                        v.wait_ge(sems["acc"], t + 1)
                        nm(t)
                    p2(3)
                    if t < NTILES:
                        v.wait_ge(sems["nmu"], t + 1)  # nm(t) committed
                        uu(t)

    return nc


def _prep(x, lora_scale_A, lora_scale_B, lora_shift_A, lora_shift_B):
    x = np.ascontiguousarray(np.asarray(x, dtype=np.float32).reshape(-1, N))
    scale = np.einsum(
        "nr,rn->n",
        np.asarray(lora_scale_A, np.float32),
        np.asarray(lora_scale_B, np.float32),
    ) * SCALING
    shift = np.einsum(
        "nr,rn->n",
        np.asarray(lora_shift_A, np.float32),
        np.asarray(lora_shift_B, np.float32),
    ) * SCALING
    args = {
        "scale_vec": np.ascontiguousarray(scale.astype(ml_dtypes.bfloat16)),
        "shift_vec": np.ascontiguousarray(shift.astype(ml_dtypes.bfloat16)),
    }
    return [
        {"x_shard": x[i * ROWS:(i + 1) * ROWS], **args} for i in range(NCORES)
    ]


def kernel(x, lora_scale_A, lora_scale_B, lora_shift_A, lora_shift_B):
    in_maps = _prep(x, lora_scale_A, lora_scale_B, lora_shift_A, lora_shift_B)
    nc = build_nc()
    res = run_bass_kernel_spmd(nc, in_maps, core_ids=list(range(NCORES)))
    out = np.concatenate(
        [res.results[i]["y_shard"] for i in range(NCORES)], axis=0
    )
    return out.reshape(B_DIM, S_DIM, N)


if __name__ == "__main__":
    import reference

    inputs = {k: np.asarray(v) for k, v in reference.setup_inputs().items()}
    expected = np.asarray(reference.reference(**inputs))
    actual = kernel(**inputs)
    err = np.abs(actual - expected)
    denom = np.abs(expected).max()
    print("max abs err:", err.max(), "rel:", err.max() / denom)
